# revision 8
# baseline (speedup 1.0000x reference)
"""Multi-head attention (Whisper-style, causal) on 8 Trainium2 cores.

Sharding: data-parallel over batch (2) x tensor-parallel over heads (4 groups
of 4 heads).  Core c handles batch c//4, heads [4*(c%4), 4*(c%4)+4).

Per-core device program (identical on all cores, data differs):
  - QKV projections as fp16x3 split matmuls (fp32-grade accuracy),
    producing qT/kT per head in [head_dim, seq] layout with an extra
    augmentation row, and v in natural [seq, head_dim] layout with an
    appended ones column (yields softmax sums for free).
  - A single-term fp16 max-pass computes per-query causal row maxima.
  - Scores are computed transposed ([k, q]) with the contraction augmented by
    a 65th row carrying (-1, max_q), so exp sees pre-shifted logits.
  - Causality is structural: only lower tiles are computed; the diagonal
    128x128 blocks get a -1e9 triangular additive mask generated on chip.
  - exp on the scalar engine (fp16 out), p@v with the ones column, reciprocal
    normalization, then the output projection in fp16.
Host combines: out[b] = sum of the 4 partials + bv @ Wo.T + bo.
"""

import numpy as np

import concourse.bass as bass
import concourse.mybir as mybir
import concourse.tile as tile
from contextlib import ExitStack
from concourse import bacc, bass_utils
from concourse.masks import make_identity, make_causal_mask

B, S, NS, H, DH = 2, 2048, 1024, 16, 64
HPC = 4                 # heads per core
CB = HPC * DH           # 256 projected columns per core
SCALE = DH ** -0.25
NEG = -1e9
P = 128
KSUB = NS // P          # 8 contraction subtiles
NSC = S // 512          # 4 s-chunks of 512
NQB = S // P            # 16 q blocks of 128
f32, f16, f32r = mybir.dt.float32, mybir.dt.float16, mybir.dt.float32r
FX = mybir.ActivationFunctionType

_PROG = None


def build_program():
    nc = bacc.Bacc("TRN2", target_bir_lowering=False, debug=False)

    xh_d = nc.dram_tensor("xh", [NS, S], f16, kind="ExternalInput").ap()
    xl_d = nc.dram_tensor("xl", [NS, S], f16, kind="ExternalInput").ap()
    wqh_d = nc.dram_tensor("wqh", [NS, CB], f16, kind="ExternalInput").ap()
    wql_d = nc.dram_tensor("wql", [NS, CB], f16, kind="ExternalInput").ap()
    wkh_d = nc.dram_tensor("wkh", [NS, CB], f16, kind="ExternalInput").ap()
    wkl_d = nc.dram_tensor("wkl", [NS, CB], f16, kind="ExternalInput").ap()
    wv_d = nc.dram_tensor("wv", [NS, CB], f16, kind="ExternalInput").ap()
    wo_d = nc.dram_tensor("wo", [CB, NS], f16, kind="ExternalInput").ap()
    sbq_d = nc.dram_tensor("sbq", [CB], f32, kind="ExternalInput").ap()
    y_d = nc.dram_tensor("y", [S, NS], f32, kind="ExternalOutput").ap()
    y_v = y_d.rearrange("(st p) j -> p st j", p=P)

    with tile.TileContext(nc) as tc, ExitStack() as stack:
        cpool = stack.enter_context(tc.tile_pool(name="cpool", bufs=1))
        wpool = stack.enter_context(tc.tile_pool(name="wpool", bufs=1))
        qkpool = stack.enter_context(tc.tile_pool(name="qkpool", bufs=1))

        # --- constants ---
        ident = cpool.tile([P, P], f32, name="ident")
        make_identity(nc, ident[:])
        tri_std = cpool.tile([P, P], f32, name="tri_std")
        make_causal_mask(nc, tri_std[:], mask_val=NEG)  # [q,k]: -1e9 where k>q
        tri_t = cpool.tile([P, P], f32, name="tri_t")   # [k,q]: -1e9 where k>q
        ones_r = cpool.tile([1, DH], f32, name="ones_r")
        nc.gpsimd.memset(ones_r[:], 1.0)

        # --- weights ---
        wq_hi = wpool.tile([P, KSUB, CB], f16, name="wq_hi")
        wq_lo = wpool.tile([P, KSUB, CB], f16, name="wq_lo")
        wk_hi = wpool.tile([P, KSUB, CB], f16, name="wk_hi")
        wk_lo = wpool.tile([P, KSUB, CB], f16, name="wk_lo")
        wv_t = wpool.tile([P, KSUB, CB], f16, name="wv_t")
        wo_t = wpool.tile([P, 2, NS], f16, name="wo_t")
        sbq_t = wpool.tile([P, 2], f32, name="sbq_t")
        for dst, src in ((wq_hi, wqh_d), (wq_lo, wql_d), (wk_hi, wkh_d),
                         (wk_lo, wkl_d), (wv_t, wv_d)):
            nc.sync.dma_start(dst[:], src.rearrange("(ko p) c -> p ko c", p=P))
        nc.sync.dma_start(wo_t[:], wo_d.rearrange("(cs p) j -> p cs j", p=P))
        nc.sync.dma_start(sbq_t[:], sbq_d.rearrange("(cs p) -> p cs", p=P))

        # tri_t = transpose(tri_std) via PE
        with tc.tile_pool(name="initps", bufs=1, space="PSUM") as initps:
            tps0 = initps.tile([P, P], f32, name="tps0")
            nc.tensor.transpose(tps0[:], tri_std[:], ident[:])
            nc.vector.tensor_copy(tri_t[:], tps0[:])

        # --- persistent activations ---
        qh = [qkpool.tile([65, S], f16, name=f"qh{h}") for h in range(HPC)]
        ql = [qkpool.tile([65, S], f16, name=f"ql{h}") for h in range(HPC)]
        kh = [qkpool.tile([65, S], f16, name=f"kh{h}") for h in range(HPC)]
        kl = [qkpool.tile([65, S], f16, name=f"kl{h}") for h in range(HPC)]
        vt = [qkpool.tile([P, NQB, DH + 1], f16, name=f"v{h}")
              for h in range(HPC)]
        oT = qkpool.tile([P, 2, S], f16, name="oT")
        mh = [qkpool.tile([P, NQB], f32, name=f"m{h}") for h in range(HPC)]

        for h in range(HPC):
            nc.gpsimd.memset(ql[h][64:65, :], 0.0)
            nc.gpsimd.memset(kl[h][64:65, :], 0.0)
            nc.gpsimd.memset(kh[h][64:65, :], -1.0)
            nc.gpsimd.memset(vt[h][:, :, DH:DH + 1], 1.0)

        # =================== Phase 1: QKV projections ===================
        with tc.tile_pool(name="xs", bufs=2) as xs, \
             tc.tile_pool(name="qkvps", bufs=2, space="PSUM") as qkvps:
            xh_v = xh_d.rearrange("(ko p) s -> p ko s", p=P)
            xl_v = xl_d.rearrange("(ko p) s -> p ko s", p=P)
            for sc in range(NSC):
                ss = slice(sc * 512, (sc + 1) * 512)
                xh_s = xs.tile([P, KSUB, 512], f16, name="xh_s")
                nc.sync.dma_start(xh_s[:], xh_v[:, :, ss])
                xl_s = xs.tile([P, KSUB, 512], f16, name="xl_s")
                nc.sync.dma_start(xl_s[:], xl_v[:, :, ss])

                for proj, w_hi, w_lo, dh_, dl_, biased in (
                        ("q", wq_hi, wq_lo, qh, ql, True),
                        ("k", wk_hi, wk_lo, kh, kl, False)):
                    for cs in range(2):
                        csl = slice(cs * P, (cs + 1) * P)
                        ps = qkvps.tile([P, 512], f32, name="qkps")
                        idx = 0
                        for wt, xt in ((w_hi, xh_s), (w_hi, xl_s),
                                       (w_lo, xh_s)):
                            for ko in range(KSUB):
                                nc.tensor.matmul(
                                    ps[:], wt[:, ko, csl], xt[:, ko, :],
                                    start=(idx == 0), stop=(idx == 23))
                                idx += 1
                        if biased:
                            nc.scalar.activation(
                                ps[:], ps[:], FX.Identity,
                                bias=sbq_t[:, cs:cs + 1])
                        for hh in range(2):
                            h = 2 * cs + hh
                            src = ps[hh * 64:(hh + 1) * 64, :]
                            nc.scalar.copy(dh_[h][0:64, ss], src)
                            nc.vector.tensor_tensor(
                                dl_[h][0:64, ss], src, dh_[h][0:64, ss],
                                mybir.AluOpType.subtract)

                # v projection for this s-chunk (4 s-tiles of 128)
                for sti in range(4):
                    st = sc * 4 + sti
                    stsl = slice(st * P, (st + 1) * P)
                    psv = qkvps.tile([P, CB], f32, name="vps")
                    for ko in range(KSUB):
                        nc.tensor.matmul(
                            psv[:], xh_s[:, ko, sti * P:(sti + 1) * P],
                            wv_t[:, ko, :],
                            start=(ko == 0), stop=(ko == KSUB - 1))
                    for h in range(HPC):
                        nc.scalar.copy(vt[h][:, st, 0:DH],
                                       psv[:, h * DH:(h + 1) * DH])

        # =================== Phase 2: causal row maxima ===================
        with tc.tile_pool(name="mxw", bufs=2) as mxw, \
             tc.tile_pool(name="mxps", bufs=2, space="PSUM") as mxps, \
             tc.tile_pool(name="tpps", bufs=1, space="PSUM") as tpps:
            for h in range(HPC):
                for qb in range(NQB):
                    nkc = qb // 4 + 1
                    for kc in range(nkc):
                        ps = mxps.tile([P, 512], f32, name="mps")
                        nc.tensor.matmul(
                            ps[:], qh[h][0:DH, qb * P:(qb + 1) * P],
                            kh[h][0:DH, kc * 512:(kc + 1) * 512],
                            start=True, stop=True)
                        if kc == nkc - 1:
                            off = qb * P - kc * 512
                            nc.vector.tensor_tensor(
                                ps[:, off:off + P], ps[:, off:off + P],
                                tri_std[:], mybir.AluOpType.add)
                            valid = off + P
                        else:
                            valid = 512
                        if kc == 0:
                            nc.vector.tensor_reduce(
                                mh[h][:, qb:qb + 1], ps[:, 0:valid],
                                axis=mybir.AxisListType.X,
                                op=mybir.AluOpType.max)
                        else:
                            tm = mxw.tile([P, 1], f32, name="tm")
                            nc.vector.tensor_reduce(
                                tm[:], ps[:, 0:valid],
                                axis=mybir.AxisListType.X,
                                op=mybir.AluOpType.max)
                            nc.vector.tensor_tensor(
                                mh[h][:, qb:qb + 1], mh[h][:, qb:qb + 1],
                                tm[:], mybir.AluOpType.max)
                # transpose maxima into the augmentation row of qh[h]
                tpm = tpps.tile([NQB, P], f32, name="tpm")
                nc.tensor.transpose(tpm[:], mh[h][:, 0:NQB], ident[:])
                mt = mxw.tile([NQB, P], f16, name="mt")
                nc.scalar.copy(mt[:], tpm[:])
                nc.sync.dma_start(qh[h][64:65, :], mt[:])

        # =================== Phase 3: scores / exp / pv ===================
        with tc.tile_pool(name="scw", bufs=4) as scw, \
             tc.tile_pool(name="nrm", bufs=2) as nrm, \
             tc.tile_pool(name="scps", bufs=3, space="PSUM") as scps, \
             tc.tile_pool(name="bcps", bufs=1, space="PSUM") as bcps, \
             tc.tile_pool(name="pvps", bufs=2, space="PSUM") as pvps:
            for h in range(HPC):
                for qc in range(NSC):
                    po = pvps.tile([DH + 1, 512], f32, name="po")
                    nkb = 4 * qc + 4
                    for kb in range(nkb):
                        j = max(0, kb - 4 * qc)
                        off = j * P
                        qs = slice(qc * 512 + off, (qc + 1) * 512)
                        ks = slice(kb * P, (kb + 1) * P)
                        ps = scps.tile([P, 512], f32, name="sps")
                        reg = ps[:, off:512]
                        nc.tensor.matmul(reg, kh[h][0:65, ks],
                                         qh[h][0:65, qs],
                                         start=True, stop=False)
                        nc.tensor.matmul(reg, kh[h][0:65, ks],
                                         ql[h][0:65, qs],
                                         start=False, stop=False)
                        nc.tensor.matmul(reg, kl[h][0:65, ks],
                                         qh[h][0:65, qs],
                                         start=False, stop=True)
                        if kb >= 4 * qc:
                            nc.vector.tensor_tensor(
                                ps[:, off:off + P], ps[:, off:off + P],
                                tri_t[:], mybir.AluOpType.add)
                        pe = scw.tile([P, 512], f16, name="pe")
                        if off:
                            nc.vector.memset(pe[:, 0:off], 0.0)
                        nc.scalar.activation(pe[:, off:512], ps[:, off:512],
                                             FX.Exp)
                        nc.tensor.matmul(po[:], vt[h][:, kb, :], pe[:],
                                         start=(kb == 0),
                                         stop=(kb == nkb - 1))
                    # normalize: rows 0..63 / row 64
                    sr = nrm.tile([1, 512], f32, name="sr")
                    nc.vector.tensor_copy(sr[:], po[DH:DH + 1, :])
                    rr = nrm.tile([1, 512], f32, name="rr")
                    nc.vector.reciprocal(rr[:], sr[:])
                    rbp = bcps.tile([DH, 512], f32, name="rbp")
                    nc.tensor.matmul(rbp[:], ones_r[:], rr[:],
                                     start=True, stop=True)
                    rb = nrm.tile([DH, 512], f32, name="rb")
                    nc.scalar.copy(rb[:], rbp[:])
                    nc.vector.tensor_tensor(
                        oT[(h % 2) * DH:(h % 2 + 1) * DH, h // 2,
                           qc * 512:(qc + 1) * 512],
                        po[0:DH, :], rb[:], mybir.AluOpType.mult)

        # =================== Phase 4: output projection ===================
        with tc.tile_pool(name="yw", bufs=4) as yw, \
             tc.tile_pool(name="yps", bufs=2, space="PSUM") as yps:
            for st in range(NQB):
                for jc in range(2):
                    jsl = slice(jc * 512, (jc + 1) * 512)
                    ps = yps.tile([P, 512], f32, name="yp")
                    for cs in range(2):
                        nc.tensor.matmul(
                            ps[:], oT[:, cs, st * P:(st + 1) * P],
                            wo_t[:, cs, jsl],
                            start=(cs == 0), stop=(cs == 1))
                    yt = yw.tile([P, 512], f32, name="yt")
                    if (st + jc) % 2:
                        nc.scalar.copy(yt[:], ps[:])
                    else:
                        nc.vector.tensor_copy(yt[:], ps[:])
                    nc.sync.dma_start(y_v[:, st, jsl], yt[:])

    nc.compile()
    return nc


def _split16(a):
    hi = a.astype(np.float16)
    lo = (a - hi.astype(np.float32)).astype(np.float16)
    return hi, lo


def _prep_core(c, x, Wq, bq, Wk, Wv, Wo):
    b, g = divmod(c, 4)
    cols = slice(g * CB, (g + 1) * CB)
    xT = np.ascontiguousarray(x[b].T).astype(np.float32)
    xh, xl = _split16(xT)
    wq = (SCALE * Wq[cols]).T.astype(np.float32)
    wqh, wql = _split16(wq)
    wk = (SCALE * Wk[cols]).T.astype(np.float32)
    wkh, wkl = _split16(wk)
    wv = Wv[cols].T.astype(np.float16)
    wo = np.ascontiguousarray(Wo[:, cols].T).astype(np.float16)
    sbq = (SCALE * bq[cols]).astype(np.float32)
    return {"xh": xh, "xl": xl, "wqh": wqh, "wql": wql, "wkh": wkh,
            "wkl": wkl, "wv": np.ascontiguousarray(wv),
            "wo": wo, "sbq": sbq}


def kernel(x, mask, Wq, bq, Wk, Wv, bv, Wo, bo):
    global _PROG
    if _PROG is None:
        _PROG = build_program()
    x = np.asarray(x, dtype=np.float32)
    in_maps = [_prep_core(c, x, np.asarray(Wq), np.asarray(bq),
                          np.asarray(Wk), np.asarray(Wv), np.asarray(Wo))
               for c in range(8)]
    res = bass_utils.run_bass_kernel_spmd(_PROG, in_maps,
                                          core_ids=list(range(8)))
    host_bias = (np.asarray(bv, np.float32) @ np.asarray(Wo, np.float32).T
                 + np.asarray(bo, np.float32))
    out = np.empty((B, S, NS), np.float32)
    for b in range(B):
        acc = res.results[4 * b]["y"].copy()
        for g in range(1, 4):
            acc += res.results[4 * b + g]["y"]
        out[b] = acc + host_bias
    return out
